# revision 1
# baseline (speedup 1.0000x reference)
"""Trainium2 Bass kernel for GQA attention block (RMSNorm-qk + RoPE + causal GQA + O-proj).

Problem shapes (hardcoded): B=2, L=2048, D=2048, H=32 q heads, HKV=8 kv heads, HD=64.

Sharding across 8 NeuronCores: 2-way data parallel on batch x 4-way tensor
parallel on heads. Core i handles batch i//4 and head-group i%4 (8 q heads,
2 kv heads — consistent with GQA grouping since group size is 4). Each core
computes its partial output (x[b] @ Wq_s ... @ Wo_s) of shape [L, D]; the host
sums the 4 partials per batch. No on-device collectives.

Per-core math layout:
  - host passes x[b] transposed (xT [D, L]) so D is the contraction partition dim
  - QKV projection into PSUM [128 tok, 512q + 256kv] via f32r matmuls
  - per-head RMSNorm: sum(q^2) per 64-wide head group, sqrt/reciprocal, scale
  - RoPE via host tables C1,S1,C2,S2 (norm weight w and softmax scale folded in)
  - PE transposes give qT [64, L] per head, kT [64, L] per kv head
  - scores computed transposed: S^T[k,q] = kT_tile.T @ qT_chunk  (PSUM [128,512])
  - exp without max subtraction (RMS-normed q,k bound |score| <= 8)
  - causal mask applied multiplicatively on the 4 diagonal k-tiles per q-chunk
  - P@V without transposing P: O^T[hd,q] accumulates Vaug_tile.T @ expS^T;
    V is augmented with a ones column so row 64 of O^T is the softmax denom
  - denom reciprocal broadcast to 64 partitions via PE outer product, folded
    into the PSUM->SBUF evacuation of attn^T
  - O-proj: out[tok, :] accumulates attnT_chunk.T @ Wo_chunk, PSUM -> DRAM
"""

import sys

import numpy as np

for _p in ("/opt/trn_rl_repo", "/root/.axon_site/_ro/trn_rl_repo"):
    if _p not in sys.path:
        sys.path.append(_p)

import concourse.bass as bass
import concourse.mybir as mybir
import concourse.tile as tile
from concourse import bacc, bass_utils
from concourse.alu_op_type import AluOpType
from concourse.masks import make_identity

F32 = mybir.dt.float32
F32R = mybir.dt.float32r
BF16 = mybir.dt.bfloat16
AF = mybir.ActivationFunctionType

# full problem shapes
B, L, D = 2, 2048, 2048
H, HKV_TOT, HD = 32, 8, 64
EPS = 1e-5
THETA = 1000000.0

N_CORES = 8
BATCH_WAYS, HEAD_WAYS = 2, 4
HQ = H // HEAD_WAYS        # 8 q heads per core
HKV = HKV_TOT // HEAD_WAYS  # 2 kv heads per core
GQ = H // HKV_TOT           # 4 q heads per kv head

P = 128
QCW = 512  # q-chunk width for attention (matmul moving dim)


def _r(x):
    return x


def build_nc(l=L, d=D, hq=HQ, hkv=HKV):
    """Build the per-core Bass program. All cores run the same program."""
    nt = l // P          # token tiles
    dc = d // P          # contraction chunks for projections
    nqc = l // QCW       # q-chunks for attention
    ktq = QCW // P       # k-tiles inside one q-chunk (diagonal band)
    fq = hq * HD         # q features per core
    fkv = hkv * HD       # kv features per core
    oc = (d + QCW - 1) // QCW  # output column chunks
    fch = fq // P        # feature chunks for O-proj contraction

    nc = bacc.Bacc("TRN2", target_bir_lowering=False, debug=False)

    xT = nc.dram_tensor("xT", [d, l], F32R, kind="ExternalInput").ap()
    wqkv = nc.dram_tensor("wqkv", [d, fq + 2 * fkv], F32R, kind="ExternalInput").ap()
    wo = nc.dram_tensor("wo", [fq, d], F32R, kind="ExternalInput").ap()
    ropeq = nc.dram_tensor("ropeq", [P, nt, 4, HD // 2], F32, kind="ExternalInput").ap()
    ropek = nc.dram_tensor("ropek", [P, nt, 4, HD // 2], F32, kind="ExternalInput").ap()
    out = nc.dram_tensor("out", [l, d], F32, kind="ExternalOutput").ap()

    with tile.TileContext(nc) as tc:
        with (
            tc.tile_pool(name="consts", bufs=1) as consts,
            tc.tile_pool(name="weights", bufs=1) as weights,
            tc.tile_pool(name="persist", bufs=1) as persist,
            tc.tile_pool(name="xin", bufs=2) as xin,
            tc.tile_pool(name="scr", bufs=1) as scr,
            tc.tile_pool(name="stat", bufs=4) as stat,
            tc.tile_pool(name="attnp", bufs=1) as attnp,
            tc.tile_pool(name="esp", bufs=1) as esp,
            tc.tile_pool(name="evacp", bufs=1) as evacp,
            tc.tile_pool(name="recp", bufs=4) as recp,
            tc.tile_pool(name="dscr", bufs=4, space="DRAM") as dscr,
            tc.tile_pool(name="ps_pq", bufs=1, space="PSUM") as ps_pq,
            tc.tile_pool(name="ps_kv", bufs=1, space="PSUM") as ps_kv_pool,
            tc.tile_pool(name="ps_sm", bufs=1, space="PSUM") as ps_sm,
            tc.tile_pool(name="ps_s", bufs=2, space="PSUM") as ps_s_pool,
            tc.tile_pool(name="ps_o", bufs=2, space="PSUM") as ps_o_pool,
        ):
            # ---------- constants ----------
            identity = consts.tile([P, P], F32)
            make_identity(nc, identity)
            ones_f32 = consts.tile([P, 1], F32)
            nc.vector.memset(ones_f32, 1.0)
            eps_sb = consts.tile([P, 1], F32)
            nc.vector.memset(eps_sb, EPS)
            # single causal mask triangle: mask[p, j] = 1.0 iff j >= p (all
            # diagonal k-tiles reduce to this after width-trimming)
            mask = consts.tile([P, QCW], F32)
            nc.vector.memset(mask, 1.0)
            nc.gpsimd.affine_select(
                out=mask, in_=mask, pattern=[[1, QCW]],
                compare_op=AluOpType.is_ge, fill=0.0, base=0,
                channel_multiplier=-1,
            )
            # ---------- x prefetch: first two tiles load before the weights ----------
            xin_next = {}
            for _t in (0, 1):
                _x = xin.tile([P, dc, P], F32R, name="x_sb", tag="x_sb", bufs=2)
                nc.sync.dma_start(
                    out=_x,
                    in_=xT.rearrange("(c p) j -> p c j", p=P)[:, :, _t * P:(_t + 1) * P],
                )
                xin_next[_t] = _x

            # ---------- weights (per-chunk DMAs so proj can start early) ----------
            wqkv_sb = weights.tile([P, dc, fq + 2 * fkv], F32R)
            for c in range(dc):
                nc.sync.dma_start(
                    out=wqkv_sb[:, c, :],
                    in_=wqkv.rearrange("(c p) j -> p c j", p=P)[:, c, :])
            rq = consts.tile([P, nt, 4, HD // 2], F32)
            nc.sync.dma_start(out=rq, in_=ropeq)
            rk = consts.tile([P, nt, 4, HD // 2], F32)
            nc.sync.dma_start(out=rk, in_=ropek)
            # wo is first needed at the first O-proj (~100us in); load it last
            wo_sb = weights.tile([P, fch, d], F32R)

            # ---------- persistent activations ----------
            # q head h -> tile h % (hq//2), partition half h // (hq//2) (same
            # half as its kv head so matmul base partitions match)
            qT = [persist.tile([P, l], F32R, name=f"qT{i}") for i in range(hq // 2)]
            kT = [persist.tile([P, l], F32R, name=f"kT{i}") for i in range(max(hkv // 2, 1))]
            vaug = persist.tile([P, nt, hkv, HD + 1], F32R)
            nc.vector.tensor_copy(
                vaug[:, :, :, HD:HD + 1],
                ones_f32.unsqueeze(2).unsqueeze(3).to_broadcast([P, nt, hkv, 1]))

            for c in range(fch):
                nc.sync.dma_start(
                    out=wo_sb[:, c, :],
                    in_=wo.rearrange("(c p) j -> p c j", p=P)[:, c, :])

            def qT_ap(h):
                t = qT[h % (hq // 2)]
                half = h // (hq // 2)
                return t[half * HD:(half + 1) * HD, :]

            def kT_ap(kv):
                t = kT[kv // 2]
                return t[(kv % 2) * HD:(kv % 2 + 1) * HD, :]

            def load_x(t):
                x_sb = xin.tile([P, dc, P], F32R, name="x_sb", tag="x_sb", bufs=2)
                nc.sync.dma_start(
                    out=x_sb,
                    in_=xT.rearrange("(c p) j -> p c j", p=P)[:, :, t * P:(t + 1) * P],
                )
                return x_sb

            def project_tile(t, x_sb):
                """QKV projection + norm + rope + transpose for token tile t."""
                ps_q = ps_pq.tile([P, fq], F32, name="ps_q", tag="pq", bufs=1)
                ps_kv = ps_kv_pool.tile([P, 2 * fkv], F32, name="ps_kv", tag="pkv", bufs=1)
                for c in range(dc):
                    nc.tensor.matmul(
                        ps_q, x_sb[:, c, :], wqkv_sb[:, c, 0:fq],
                        start=(c == 0), stop=(c == dc - 1),
                    )
                for c in range(dc):
                    nc.tensor.matmul(
                        ps_kv, x_sb[:, c, :], wqkv_sb[:, c, fq:fq + 2 * fkv],
                        start=(c == 0), stop=(c == dc - 1),
                    )

                groups = [(ps_q, hq, rq, qT_ap), (ps_kv[:, 0:fkv], hkv, rk, kT_ap)]
                invs = []
                sqs = []
                for (ps, nh, rt, dstT) in groups:
                    psg = ps.rearrange("p (h e) -> p h e", e=HD)
                    sq = scr.tile([P, nh, HD], F32, name="sq", tag="nsc", bufs=4)
                    nc.scalar.activation(sq, psg, AF.Square)
                    sqs.append(sq)
                sds = []
                for (ps, nh, rt, dstT), sq in zip(groups, sqs):
                    ss = stat.tile([P, nh], F32, name="ss", tag="ss")
                    nc.vector.reduce_sum(out=ss, in_=sq, axis=mybir.AxisListType.X)
                    sd = stat.tile([P, nh], F32, name="sd", tag="sd")
                    nc.scalar.activation(sd, ss, AF.Sqrt, scale=1.0 / HD, bias=eps_sb)
                    sds.append(sd)
                for (ps, nh, rt, dstT), sd in zip(groups, sds):
                    psg = ps.rearrange("p (h e) -> p h e", e=HD)
                    inv = stat.tile([P, nh], F32, name="inv", tag="inv")
                    nc.vector.reciprocal(inv, sd)
                    qn = scr.tile([P, nh, HD], F32, name="qn", tag="nsc", bufs=4)
                    nc.vector.tensor_mul(
                        qn, psg, inv.unsqueeze(2).to_broadcast([P, nh, HD]))
                    qr = scr.tile([P, nh, HD], F32, name="qr", tag="nsc", bufs=4)
                    tmp = scr.tile([P, nh, HD // 2], F32, name="tmp", tag="tmp", bufs=2)
                    hw = HD // 2

                    def tab(i):
                        return rt[:, t, i, :].unsqueeze(1).to_broadcast([P, nh, hw])

                    # out1 = q1*C1 - q2*S2 ; out2 = q2*C2 + q1*S1
                    nc.vector.tensor_mul(qr[:, :, 0:hw], qn[:, :, 0:hw], tab(0))
                    nc.vector.tensor_mul(tmp, qn[:, :, hw:HD], tab(3))
                    nc.vector.tensor_sub(qr[:, :, 0:hw], qr[:, :, 0:hw], tmp)
                    nc.vector.tensor_mul(qr[:, :, hw:HD], qn[:, :, hw:HD], tab(2))
                    nc.vector.tensor_mul(tmp, qn[:, :, 0:hw], tab(1))
                    nc.vector.tensor_add(qr[:, :, hw:HD], qr[:, :, hw:HD], tmp)

                    for h in range(nh):
                        ps_t = ps_sm.tile([HD, P], F32, name="ps_t", tag="psm", bufs=1)
                        nc.tensor.transpose(ps_t, qr[:, h, :], identity)
                        if h % 2 == 0:
                            nc.scalar.copy(dstT(h)[:, t * P:(t + 1) * P], ps_t)
                        else:
                            nc.vector.tensor_copy(dstT(h)[:, t * P:(t + 1) * P], ps_t)

                for kv in range(hkv):
                    nc.scalar.copy(
                        vaug[:, t, kv, 0:HD],
                        ps_kv[:, fkv + kv * HD:fkv + (kv + 1) * HD],
                    )

            # ============ fused per-q-chunk pipeline: project -> attend -> O-proj ============
            def project_chunk(cq):
                for t in range(cq * ktq, (cq + 1) * ktq):
                    x_sb = xin_next.pop(t, None)
                    if x_sb is None:
                        x_sb = load_x(t)
                    if t + 1 < nt and (t + 1) not in xin_next:
                        xin_next[t + 1] = load_x(t + 1)
                    project_tile(t, x_sb)

            project_chunk(0)
            for qc in range(nqc):
                # emit next chunk's projection before this chunk's attention so
                # the static schedule overlaps PE-heavy proj with ACT-heavy attn
                if qc + 1 < nqc:
                    project_chunk(qc + 1)

                attnT = attnp.tile([P, fq // P, QCW], F32R, name="attnT", tag="attnT", bufs=1)
                nkt = (qc + 1) * ktq
                for kv in range(hkv):
                    for hl in range(GQ):
                        h = kv * GQ + hl
                        ps_o = ps_o_pool.tile([HD + 1, QCW], F32, name="ps_o", tag="po", bufs=2)
                        for kt in range(nkt):
                            dgl = kt - qc * ktq
                            # width-trim diagonal tiles: columns [w0, QCW) valid
                            w0 = max(dgl, 0) * P
                            n = QCW - w0
                            qslice = qT_ap(h)[:, qc * QCW + w0:(qc + 1) * QCW]
                            ps_s = ps_s_pool.tile([P, QCW], F32, name="ps_s", tag="ps", bufs=2)
                            nc.tensor.matmul(
                                ps_s[:, 0:n], kT_ap(kv)[:, kt * P:(kt + 1) * P],
                                qslice, start=True, stop=True,
                            )
                            es = esp.tile([P, QCW], F32R, name="es", tag="es", bufs=4)
                            nc.scalar.activation(es[:, 0:n], ps_s[:, 0:n], AF.Exp)
                            if dgl >= 0:
                                nc.vector.tensor_mul(
                                    es[:, 0:n], es[:, 0:n], mask[:, 0:n])
                            nc.tensor.matmul(
                                ps_o[:, w0:QCW], vaug[:, kt, kv, :], es[:, 0:n],
                                start=(kt == 0), stop=(kt == nkt - 1),
                            )
                        rec = recp.tile([1, QCW], F32, name="rec", tag="rec")
                        nc.vector.reciprocal(rec, ps_o[HD:HD + 1, :])
                        recd = dscr.tile([1, QCW], F32, name="recd", tag="recd")
                        nc.sync.dma_start(out=recd, in_=rec)
                        rb = evacp.tile([HD, QCW], F32, name="rb", tag="evac", bufs=3)
                        nc.sync.dma_start(
                            out=rb, in_=recd.partition_broadcast(HD).squeeze(1))
                        nc.vector.tensor_mul(
                            attnT[(h % 2) * HD:(h % 2 + 1) * HD, h // 2, :],
                            ps_o[0:HD, :], rb,
                        )
                # O-proj for this q-chunk
                for tt in range(ktq):
                    row0 = qc * QCW + tt * P
                    for ncol in range(oc):
                        ps_out = ps_pq.tile([P, QCW], F32, name="ps_out", tag="pout", bufs=1)
                        for fc in range(fch):
                            nc.tensor.matmul(
                                ps_out,
                                attnT[:, fc, tt * P:(tt + 1) * P],
                                wo_sb[:, fc, ncol * QCW:(ncol + 1) * QCW],
                                start=(fc == 0), stop=(fc == fch - 1),
                            )
                        ost = evacp.tile([P, QCW], F32, name="ost", tag="evac", bufs=3)
                        nc.vector.tensor_copy(ost, ps_out)
                        nc.sync.dma_start(
                            out=out[row0:row0 + P, ncol * QCW:(ncol + 1) * QCW],
                            in_=ost,
                        )
    nc.compile()
    return nc


def make_rope_tables(norm_w, scale, l, nt):
    """Pack [P, nt, 4, 32] tables: C1=cos*w1*s, S1=sin*w1*s, C2=cos*w2*s, S2=sin*w2*s."""
    half = HD // 2
    inv_freq = THETA ** (-np.arange(0, HD, 2, dtype=np.float32) / HD)
    ang = np.arange(l, dtype=np.float32)[:, None] * inv_freq[None, :]
    cos, sin = np.cos(ang), np.sin(ang)  # [l, 32]
    w1 = norm_w[:half].astype(np.float32) * scale
    w2 = norm_w[half:].astype(np.float32) * scale
    tabs = np.stack([cos * w1, sin * w1, cos * w2, sin * w2], axis=1)  # [l, 4, 32]
    return np.ascontiguousarray(
        tabs.reshape(nt, P, 4, half).transpose(1, 0, 2, 3)).astype(np.float32)


def make_in_maps(x, Wq, Wk, Wv, Wo, q_norm_w, k_norm_w, l=L, d=D):
    nt = l // P
    scale = HD ** -0.5
    rq = make_rope_tables(np.asarray(q_norm_w), scale, l, nt)
    rk = make_rope_tables(np.asarray(k_norm_w), 1.0, l, nt)
    in_maps = []
    for i in range(N_CORES):
        b, g = i // HEAD_WAYS, i % HEAD_WAYS
        fq, fkv = HQ * HD, HKV * HD
        wq_s = Wq[:, g * fq:(g + 1) * fq]
        wk_s = Wk[:, g * fkv:(g + 1) * fkv]
        wv_s = Wv[:, g * fkv:(g + 1) * fkv]
        in_maps.append({
            "xT": np.ascontiguousarray(np.asarray(x[b], np.float32).T),
            "wqkv": np.ascontiguousarray(
                np.concatenate([wq_s, wk_s, wv_s], axis=1), dtype=np.float32),
            "wo": np.ascontiguousarray(Wo[g * fq:(g + 1) * fq, :], dtype=np.float32),
            "ropeq": rq,
            "ropek": rk,
        })
    return in_maps


def kernel(x, Wq, Wk, Wv, Wo, q_norm_w, k_norm_w):
    x = np.asarray(x, np.float32)
    in_maps = make_in_maps(x, np.asarray(Wq, np.float32), np.asarray(Wk, np.float32),
                           np.asarray(Wv, np.float32), np.asarray(Wo, np.float32),
                           np.asarray(q_norm_w, np.float32),
                           np.asarray(k_norm_w, np.float32))
    nc = build_nc()
    res = bass_utils.run_bass_kernel_spmd(nc, in_maps, core_ids=list(range(N_CORES)))
    outs = [r["out"] for r in res.results]
    full = np.empty((B, L, D), dtype=np.float32)
    for b in range(BATCH_WAYS):
        full[b] = np.sum(outs[b * HEAD_WAYS:(b + 1) * HEAD_WAYS], axis=0)
    return full



# revision 54
# speedup vs baseline: 1.2449x; 1.2449x over previous
"""Trainium2 Bass kernel for GQA attention block (RMSNorm-qk + RoPE + causal GQA + O-proj).

Problem shapes (hardcoded): B=2, L=2048, D=2048, H=32 q heads, HKV=8 kv heads, HD=64.

Sharding across 8 NeuronCores: 2-way data parallel on batch x 4-way tensor
parallel on heads. Core i handles batch i//4 and head-group i%4 (8 q heads,
2 kv heads). Each core computes its partial output (x[b] @ Wq_s ... @ Wo_s)
of shape [L, D]; the host sums the 4 partials per batch.

Per-core layout / schedule:
  - all activations bf16 (60x rel-err headroom vs the 2e-2 gate), PSUM f32
  - QKV proj: x stationary [dchunk, tok], W moving -> ps_q [tok, 512] /
    ps_kv [tok, 256]
  - RMSNorm inv = exp(-0.5*ln(mean+eps)) so ACT only ever uses
    {Square, Ln, Exp, Copy} = one act-function set -> no table reloads
  - RoPE on DVE in bf16 (2x mode); q heads host-permuted [0,4,1,5,2,6,3,7]
    so the 128-row XBAR DMA-transpose emits qT blocks whose partition halves
    line up with their kv head (no PE transposes at all)
  - attention S^T = kT.T @ qT per 128-key tile into grouped PSUM [P,1024];
    one Exp per group (scale=1/8 folded in); causal diagonal tiles packed
    (512+384 | 256+128) and masked multiplicatively in bf16
  - P@V accumulates vaug.T @ es into ps_o [65, n]; row 64 = softmax denom
    (ones column in vaug); 1/denom broadcast to 64 partitions via a PE outer
    product with a ones stationary, then folded into the PSUM evacuation
  - O-proj: attnT (f32r) stationary, Wo (f32r) moving; wide [P,1024] units
    evacuated alternately on DVE/ACT, one output-row-tile DMA per token tile
  - single preloaded ACT table set; weight/output DMAs on the ACT hwdge
    queue, x stream + XBAR transposes on SP
  - software pipeline per chunk: slot h emits [proj tile of chunk+1] ->
    scores+exp(h) -> PV(h-1)+normalize -> O-proj unit of chunk-1, so PE
    always has matmul work while ACT drains the exps
"""

import sys

import numpy as np

for _p in ("/opt/trn_rl_repo", "/root/.axon_site/_ro/trn_rl_repo"):
    if _p not in sys.path:
        sys.path.append(_p)

import ml_dtypes

import concourse.bass as bass
import concourse.mybir as mybir
import concourse.tile as tile
from concourse import bacc, bass_utils

F32 = mybir.dt.float32
F32R = mybir.dt.float32r
BF16 = mybir.dt.bfloat16
AF = mybir.ActivationFunctionType
AX = mybir.AxisListType
BFNP = ml_dtypes.bfloat16

# full problem shapes
B, L, D = 2, 2048, 2048
H, HKV_TOT, HD = 32, 8, 64
EPS = 1e-5
THETA = 1000000.0

N_CORES = 8
BATCH_WAYS, HEAD_WAYS = 2, 4
HQ = H // HEAD_WAYS          # 8 q heads per core
HKV = HKV_TOT // HEAD_WAYS   # 2 kv heads per core
GQ = HQ // HKV               # 4 q heads per kv head

P = 128
QCW = 512                    # q-chunk width
HW = HD // 2                 # rope half
QPERM = [0, 4, 1, 5, 2, 6, 3, 7]  # host-side q head order (and Wo row order)

NT = L // P                  # 16 token tiles
DC = D // P                  # 16 contraction chunks
NQC = L // QCW               # 4 q chunks
KTQ = QCW // P               # 4 k tiles per q chunk
FQ = HQ * HD                 # 512
FKV = HKV * HD               # 128
SCALE = HD ** -0.5

# es column layout per (head, chunk qc): non-diag kt pairs at 1024*g, then
# packed diag groups A (512+384) and B (256+128)
ES_W = QCW * KTQ * (NQC - 1) + 1280   # 7424

INTERLEAVE = False    # interleave PV matmul pairs between score groups
PROJ_IN_RING = False  # proj PSUM from shared ring (else dedicated pq/pkv banks)
RING_BUFS = 2         # shared-ring depth (3 only if PROJ_IN_RING)
OU_NARROW = False     # 16 narrow O-proj units/chunk (else 8 merged wide)
X_BUFS = 4            # x tile ring depth


def es_slice(qc, kt):
    """(offset, width, w0) of k-tile kt inside the es tile for chunk qc."""
    nd = qc * KTQ
    if kt < nd:
        return 1024 * (kt // 2) + QCW * (kt % 2), QCW, 0
    d = kt - nd
    base = KTQ * QCW * qc
    offs = (0, QCW, QCW + 384, QCW + 384 + 256)
    return base + offs[d], QCW - P * d, P * d


def _act_set_id(nc):
    """Index of the act-function set containing everything we use, so one
    preloaded table covers the whole kernel (no mid-kernel reloads)."""
    from concourse.hw_specs import get_activation_tables
    need = {AF.Exp, AF.Ln, AF.Square, AF.Copy, AF.Identity}
    for idx, (name, funcs) in enumerate(get_activation_tables(nc.m.arch).items()):
        if need <= funcs:
            return idx
    raise AssertionError("no act function set covers Exp+Ln+Square+Copy")


def build_nc():
    nc = bacc.Bacc("TRN2", target_bir_lowering=False, debug=False)

    xt = nc.dram_tensor("xt", [NT, P, DC, P], BF16, kind="ExternalInput").ap()
    wqkv = nc.dram_tensor("wqkv", [D, FQ + 2 * FKV], BF16, kind="ExternalInput").ap()
    wo = nc.dram_tensor("wo", [FQ, D], F32R, kind="ExternalInput").ap()
    ropeq = nc.dram_tensor("ropeq", [P, NT, 4, HW], BF16, kind="ExternalInput").ap()
    ropek = nc.dram_tensor("ropek", [P, NT, 4, HW], BF16, kind="ExternalInput").ap()
    out = nc.dram_tensor("out", [L, D], F32, kind="ExternalOutput").ap()

    with tile.TileContext(nc) as tc:
        with (
            tc.tile_pool(name="consts", bufs=1) as consts,
            tc.tile_pool(name="weights", bufs=1) as weights,
            tc.tile_pool(name="persist", bufs=1) as persist,
            tc.tile_pool(name="xin", bufs=2) as xin,
            tc.tile_pool(name="nrm", bufs=2) as nrm,
            tc.tile_pool(name="esb", bufs=2) as esb,
            tc.tile_pool(name="attnp", bufs=2) as attnp,
            tc.tile_pool(name="evacp", bufs=1) as evacp,
            tc.tile_pool(name="ps_s", bufs=RING_BUFS, space="PSUM") as ps_s_pool,
            tc.tile_pool(name="ps_proj", bufs=1, space="PSUM") as ps_proj,
            tc.tile_pool(name="ps_o", bufs=2, space="PSUM") as ps_o_pool,
        ):
            # ---------- constants ----------
            nc.scalar.add_instruction(mybir.InstLoadActFuncSet(
                name=nc.get_next_instruction_name(), ins=[], outs=[],
                act_func_set_id=_act_set_id(nc)))
            eps_sb = consts.tile([P, 1], F32)
            nc.vector.memset(eps_sb, EPS)
            ones_bf = consts.tile([P, 1], BF16)
            nc.vector.memset(ones_bf, 1.0)
            ones_st = consts.tile([1, HD], BF16)
            nc.vector.memset(ones_st, 1.0)
            # packed diagonal masks: per 128-key tile, column j valid iff j >= p
            maskA = consts.tile([P, QCW + 384], BF16)
            maskB = consts.tile([P, 256 + P], BF16)
            for m, ws in ((maskA, (QCW, 384)), (maskB, (256, P))):
                nc.vector.memset(m, 1.0)
                off = 0
                for w in ws:
                    nc.gpsimd.affine_select(
                        out=m[:, off:off + w], in_=m[:, off:off + w],
                        pattern=[[1, w]], compare_op=mybir.AluOpType.is_ge,
                        fill=0.0, base=0, channel_multiplier=-1,
                    )
                    off += w

            # ---------- x prefetch ----------
            xin_next = {}

            def load_x(t):
                x_sb = xin.tile([P, DC, P], BF16, name="x_sb", tag="x_sb", bufs=X_BUFS)
                nc.sync.dma_start(out=x_sb, in_=xt[t])
                return x_sb

            # weight/table loads go through the ACT hwdge queue so the
            # latency-critical x stream owns the SP queue
            wqkv_sb = weights.tile([P, DC, FQ + 2 * FKV], BF16)
            nc.scalar.dma_start(
                out=wqkv_sb[:, 0, :],
                in_=wqkv.rearrange("(c p) j -> p c j", p=P)[:, 0, :])
            for _t in range(min(X_BUFS, 4)):
                xin_next[_t] = load_x(_t)
            for c in range(1, DC):
                nc.scalar.dma_start(
                    out=wqkv_sb[:, c, :],
                    in_=wqkv.rearrange("(c p) j -> p c j", p=P)[:, c, :])
            rq = consts.tile([P, NT, 4, HW], BF16)
            nc.scalar.dma_start(out=rq, in_=ropeq)
            rk = consts.tile([P, NT, 4, HW], BF16)
            nc.scalar.dma_start(out=rk, in_=ropek)

            # ---------- persistent activations ----------
            # qT block j holds q heads j (parts 0:64) and j+4 (parts 64:128)
            qT = persist.tile([P, HQ // 2, L], BF16)
            kT = persist.tile([P, L], BF16)  # kv0 parts 0:64, kv1 parts 64:128
            vaug = persist.tile([P, NT, HKV, HD + 1], BF16)
            nc.vector.tensor_copy(
                vaug[:, :, :, HD:HD + 1],
                ones_bf.unsqueeze(2).unsqueeze(3).to_broadcast([P, NT, HKV, 1]))

            # wo needed first ~100us in; DMAs are emitted mid-prologue (below)
            wo_sb = weights.tile([P, FQ // P, D], F32R)

            def load_wo():
                for c in range(FQ // P):
                    nc.scalar.dma_start(
                        out=wo_sb[:, c, :],
                        in_=wo.rearrange("(c p) j -> p c j", p=P)[:, c, :])

            def q_ap(h):
                off = 0 if h < GQ else HD
                return qT[off:off + HD, h % GQ, :]

            def k_ap(kv):
                return kT[kv * HD:(kv + 1) * HD, :]

            # ---------- projection + norm + rope for one token tile ----------
            def proj_tile(t):
                x_sb = xin_next.pop(t, None)
                if x_sb is None:
                    x_sb = load_x(t)
                for tn in (t + 1, t + 2):
                    if tn < NT and tn not in xin_next:
                        xin_next[tn] = load_x(tn)

                if PROJ_IN_RING:
                    ps_q = ps_s_pool.tile(
                        [P, 1024], F32, name="ps_q", tag="ps",
                        bufs=RING_BUFS)[:, 0:FQ]
                else:
                    ps_q = ps_proj.tile([P, FQ], F32, name="ps_q", tag="pq", bufs=1)
                for c in range(DC):
                    nc.tensor.matmul(
                        ps_q, x_sb[:, c, :], wqkv_sb[:, c, 0:FQ],
                        start=(c == 0), stop=(c == DC - 1))
                if PROJ_IN_RING:
                    ps_kv = ps_s_pool.tile(
                        [P, 1024], F32, name="ps_kv", tag="ps",
                        bufs=RING_BUFS)[:, 0:2 * FKV]
                else:
                    ps_kv = ps_proj.tile(
                        [P, 2 * FKV], F32, name="ps_kv", tag="pkv", bufs=1)
                for c in range(DC):
                    nc.tensor.matmul(
                        ps_kv, x_sb[:, c, :], wqkv_sb[:, c, FQ:FQ + 2 * FKV],
                        start=(c == 0), stop=(c == DC - 1))

                psq_g = ps_q.rearrange("p (h e) -> p h e", e=HD)
                psk_g = ps_kv[:, 0:FKV].rearrange("p (h e) -> p h e", e=HD)
                nh = HQ + HKV  # 10 normed heads (q then k)

                sq = nrm.tile([P, nh, HD], BF16, name="sq", tag="sq", bufs=2)
                nc.scalar.activation(sq[:, 0:HQ, :], psq_g, AF.Square)
                nc.scalar.activation(sq[:, HQ:nh, :], psk_g, AF.Square)
                ss = nrm.tile([P, nh], F32, name="ss", tag="ss", bufs=2)
                nc.vector.reduce_sum(out=ss, in_=sq, axis=AX.X)
                lnt = nrm.tile([P, nh], F32, name="lnt", tag="lnt", bufs=2)
                nc.scalar.activation(lnt, ss, AF.Ln, scale=1.0 / HD, bias=eps_sb)
                inv = nrm.tile([P, nh], F32, name="inv", tag="inv", bufs=2)
                nc.scalar.activation(inv, lnt, AF.Exp, scale=-0.5)

                qn = nrm.tile([P, nh, HD], BF16, name="qn", tag="qn", bufs=2)
                nc.vector.tensor_mul(
                    qn[:, 0:HQ, :], psq_g,
                    inv[:, 0:HQ].unsqueeze(2).to_broadcast([P, HQ, HD]))
                nc.vector.tensor_mul(
                    qn[:, HQ:nh, :], psk_g,
                    inv[:, HQ:nh].unsqueeze(2).to_broadcast([P, HKV, HD]))

                qr = nrm.tile([P, nh, HD], BF16, name="qr", tag="qr", bufs=2)
                tmp = nrm.tile([P, nh, HW], BF16, name="tmp", tag="tmp", bufs=2)
                for (a, b, rt) in ((0, HQ, rq), (HQ, nh, rk)):
                    n = b - a

                    def tab(i):
                        return rt[:, t, i, :].unsqueeze(1).to_broadcast([P, n, HW])

                    q1, q2 = qn[:, a:b, 0:HW], qn[:, a:b, HW:HD]
                    r1, r2 = qr[:, a:b, 0:HW], qr[:, a:b, HW:HD]
                    tm = tmp[:, a:b, :]
                    # r1 = q1*C1 - q2*S2 ; r2 = q2*C2 + q1*S1
                    nc.vector.tensor_mul(r1, q1, tab(0))
                    nc.vector.tensor_mul(tm, q2, tab(3))
                    nc.vector.tensor_sub(r1, r1, tm)
                    nc.vector.tensor_mul(r2, q2, tab(2))
                    nc.vector.tensor_mul(tm, q1, tab(1))
                    nc.vector.tensor_add(r2, r2, tm)

                # XBAR transposes: qr q-part [128,512] -> qT[:, :, t*128:+128]
                nc.sync.dma_start_transpose(
                    qT[:, :, t * P:(t + 1) * P], qr[:, 0:HQ, :])
                nc.sync.dma_start_transpose(
                    kT[:, t * P:(t + 1) * P], qr[:, HQ:nh, :])
                # V (un-normed, un-roped)
                nc.scalar.copy(
                    vaug[:, t, :, 0:HD],
                    ps_kv[:, FKV:2 * FKV].rearrange("p (h e) -> p h e", e=HD))

            # ---------- attention: scores(h) interleaved with P@V(h-1) ----------
            # PE emits score-groups at ~0.4us while ACT drains each exp in
            # ~0.9us; raw emission outruns the 3-deep PSUM ring. Interleaving
            # PV matmul pairs of the previous head between score groups
            # rate-matches PE to ACT.
            def score_groups(qc, h, es):
                qs = q_ap(h)
                ks = k_ap(h // GQ)
                nd = qc * KTQ
                q0 = qc * QCW
                base = KTQ * QCW * qc

                def mm(ps, pcol, kt, qoff, w, start, stop):
                    nc.tensor.matmul(
                        ps[:, pcol:pcol + w],
                        ks[:, kt * P:(kt + 1) * P],
                        qs[:, q0 + qoff:q0 + qoff + w],
                        start=start, stop=stop)

                def g_nd(g):
                    def f():
                        ps = ps_s_pool.tile(
                            [P, 1024], F32, name="ps_s", tag="ps", bufs=RING_BUFS)
                        mm(ps, 0, 2 * g, 0, QCW, True, True)
                        mm(ps, QCW, 2 * g + 1, 0, QCW, True, True)
                        off = 1024 * g
                        nc.scalar.activation(
                            es[:, off:off + 1024], ps[:, 0:1024], AF.Exp,
                            scale=SCALE)
                    return f

                def g_diag(dgl):
                    def f():
                        ps = ps_s_pool.tile(
                            [P, 1024], F32, name="ps_s", tag="ps", bufs=RING_BUFS)
                        if dgl == 0:  # A = (d0 512 | d1 384)
                            mm(ps, 0, nd, 0, QCW, True, True)
                            mm(ps, QCW, nd + 1, P, 384, True, True)
                            eo, w, mask = base, 896, maskA
                        else:         # B = (d2 256 | d3 128)
                            mm(ps, 0, nd + 2, 2 * P, 256, True, True)
                            mm(ps, 256, nd + 3, 3 * P, P, True, True)
                            eo, w, mask = base + 896, 384, maskB
                        nc.scalar.activation(
                            es[:, eo:eo + w], ps[:, 0:w], AF.Exp, scale=SCALE)
                        nc.vector.tensor_mul(
                            es[:, eo:eo + w], es[:, eo:eo + w], mask)
                    return f

                return [g_nd(g) for g in range(nd // 2)] + [g_diag(0), g_diag(1)]

            def pv_pairs(qc, h, es, ps_o):
                kv = h // GQ
                nkt = (qc + 1) * KTQ

                def pair(k0):
                    def f():
                        for kt in (k0, k0 + 1):
                            off, w, w0 = es_slice(qc, kt)
                            nc.tensor.matmul(
                                ps_o[:, w0:QCW], vaug[:, kt, kv, :],
                                es[:, off:off + w],
                                start=(kt == 0), stop=(kt == nkt - 1))
                    return f

                return [pair(k0) for k0 in range(0, nkt, 2)]

            def pv_rec(ps_o):
                rec = evacp.tile([1, QCW], BF16, name="rec", tag="rec", bufs=4)
                with nc.allow_low_precision("1/denom in bf16; 2e-2 rel-err budget"):
                    nc.vector.reciprocal(rec, ps_o[HD:HD + 1, :])
                return rec

            def pv_finish(h, ps_o, rec, attnT):
                # broadcast 1/denom to 64 partitions via PE outer product
                ps_r = ps_s_pool.tile([P, 1024], F32, name="ps_r", tag="ps", bufs=RING_BUFS)
                nc.tensor.matmul(
                    ps_r[0:HD, 0:QCW], ones_st, rec, start=True, stop=True)
                rb = evacp.tile([HD, QCW], F32, name="rb", tag="rb", bufs=4)
                nc.vector.tensor_copy(rb, ps_r[0:HD, 0:QCW])
                off = 0 if h < GQ else HD
                nc.vector.tensor_mul(
                    attnT[off:off + HD, h % GQ, :], ps_o[0:HD, :], rb)

            def attn_slot(qc, h, es_pv, attnT=None, interleave=INTERLEAVE):
                """scores+exp head h; PV+evac of head h-1."""
                es = esb.tile([P, ES_W], BF16, name="es", tag="es", bufs=2)
                sc = score_groups(qc, h, es)
                pv = []
                ps_o = None
                if es_pv is not None:
                    ps_o = ps_o_pool.tile(
                        [HD + 1, QCW], F32, name="ps_o", tag="po", bufs=2)
                    pv = pv_pairs(qc, h - 1, es_pv, ps_o)
                if interleave:
                    order = sc[0:2] + [x for p in zip(pv, sc[2:]) for x in p]
                    n = min(len(pv), len(sc) - 2)
                    order += pv[n:] + sc[2 + n:]
                else:
                    order = sc + pv
                for f in order:
                    f()
                if es_pv is not None:
                    pv_finish(h - 1, ps_o, pv_rec(ps_o), attnT)
                return es

            def pv_head(qc, h, es):
                ps_o = ps_o_pool.tile(
                    [HD + 1, QCW], F32, name="ps_o", tag="po", bufs=2)
                for f in pv_pairs(qc, h, es, ps_o):
                    f()
                return (h, ps_o, pv_rec(ps_o))

            # ---------- normalize into attnT, two slots later (Pool, SBUF-only) ----
            # ---------- O-projection unit: one token tile, two column chunks ----------
            ost_cur = [None]

            def oproj_unit(qc, u, attnT):
                if OU_NARROW:
                    # u in 0..15 per chunk: one column chunk per unit
                    tt, col = u // KTQ, u % KTQ
                    ps = ps_s_pool.tile(
                        [P, 1024], F32, name="ps_s", tag="ps", bufs=RING_BUFS)
                    for fc in range(FQ // P):
                        nc.tensor.matmul(
                            ps[:, 0:QCW],
                            attnT[:, fc, tt * P:(tt + 1) * P],
                            wo_sb[:, fc, col * QCW:(col + 1) * QCW],
                            start=(fc == 0), stop=(fc == FQ // P - 1))
                    ost = evacp.tile([P, QCW], F32, name="ostn", tag="ostn", bufs=3)
                    if u % 2 == 0:
                        nc.vector.tensor_copy(ost, ps[:, 0:QCW])
                    else:
                        nc.scalar.copy(ost, ps[:, 0:QCW])
                    row0 = qc * QCW + tt * P
                    nc.scalar.dma_start(
                        out=out[row0:row0 + P, col * QCW:(col + 1) * QCW],
                        in_=ost)
                    return
                # u in 0..7 per chunk: token tile u//2, column pair u%2
                tt, cp = u // 2, u % 2
                ps = ps_s_pool.tile([P, 1024], F32, name="ps_s", tag="ps", bufs=RING_BUFS)
                for half in range(2):
                    col = 2 * cp + half
                    for fc in range(FQ // P):
                        nc.tensor.matmul(
                            ps[:, half * QCW:(half + 1) * QCW],
                            attnT[:, fc, tt * P:(tt + 1) * P],
                            wo_sb[:, fc, col * QCW:(col + 1) * QCW],
                            start=(fc == 0), stop=(fc == FQ // P - 1))
                if cp == 0:
                    ost_cur[0] = evacp.tile([P, D], F32, name="ost", tag="ost", bufs=2)
                ost = ost_cur[0]
                if u % 2 == 0:
                    nc.vector.tensor_copy(ost[:, 0:2 * QCW], ps)
                else:
                    nc.scalar.copy(ost[:, 2 * QCW:4 * QCW], ps)
                if cp == 1:
                    row0 = qc * QCW + tt * P
                    nc.scalar.dma_start(out=out[row0:row0 + P, :], in_=ost)

            # ---------- software-pipelined schedule ----------
            for t in range(KTQ):
                proj_tile(t)
            attnT_cur = None
            attnT_prev = None
            for qc in range(NQC):
                attnT_prev = attnT_cur
                attnT_cur = attnp.tile(
                    [P, FQ // P, QCW], F32R, name="attnT", tag="attnT", bufs=2)
                es_prev = None
                for h in range(HQ):
                    if qc == 0 and h == 2:
                        load_wo()
                    if qc + 1 < NQC and h % 2 == 0:
                        proj_tile(KTQ * (qc + 1) + h // 2)
                    es_prev = attn_slot(qc, h, es_prev, attnT_cur)
                    if qc > 0:
                        if OU_NARROW:
                            oproj_unit(qc - 1, 2 * h, attnT_prev)
                            oproj_unit(qc - 1, 2 * h + 1, attnT_prev)
                        else:
                            oproj_unit(qc - 1, h, attnT_prev)
                h, ps_o, rec = pv_head(qc, HQ - 1, es_prev)
                pv_finish(h, ps_o, rec, attnT_cur)
            for u in range((4 if OU_NARROW else 2) * KTQ):
                oproj_unit(NQC - 1, u, attnT_cur)

    nc.compile()
    return nc


def make_rope_tables(norm_w, l, nt):
    """[P, nt, 4, 32] bf16: C1=cos*w1, S1=sin*w1, C2=cos*w2, S2=sin*w2."""
    inv_freq = THETA ** (-np.arange(0, HD, 2, dtype=np.float64) / HD)
    ang = np.arange(l, dtype=np.float64)[:, None] * inv_freq[None, :]
    cos, sin = np.cos(ang), np.sin(ang)  # [l, 32]
    w1 = norm_w[:HW].astype(np.float64)
    w2 = norm_w[HW:].astype(np.float64)
    tabs = np.stack([cos * w1, sin * w1, cos * w2, sin * w2], axis=1)  # [l,4,32]
    return np.ascontiguousarray(
        tabs.reshape(nt, P, 4, HW).transpose(1, 0, 2, 3)).astype(BFNP)


def make_in_maps(x, Wq, Wk, Wv, Wo, q_norm_w, k_norm_w):
    rq = make_rope_tables(np.asarray(q_norm_w), L, NT)
    rk = make_rope_tables(np.asarray(k_norm_w), L, NT)
    Wq = np.asarray(Wq, np.float32).reshape(D, H, HD)
    Wo = np.asarray(Wo, np.float32).reshape(H, HD, D)
    in_maps = []
    for i in range(N_CORES):
        b, g = i // HEAD_WAYS, i % HEAD_WAYS
        heads = [g * HQ + p for p in QPERM]
        wq_s = Wq[:, heads, :].reshape(D, FQ)
        wk_s = np.asarray(Wk, np.float32)[:, g * FKV:(g + 1) * FKV]
        wv_s = np.asarray(Wv, np.float32)[:, g * FKV:(g + 1) * FKV]
        xb = np.asarray(x[b], np.float32).astype(BFNP)
        # xt[t, p, c, j] = x[b][t*128 + j, c*128 + p]
        xtl = np.ascontiguousarray(
            xb.reshape(NT, P, DC, P).transpose(0, 3, 2, 1))
        in_maps.append({
            "xt": xtl,
            "wqkv": np.ascontiguousarray(
                np.concatenate([wq_s, wk_s, wv_s], axis=1)).astype(BFNP),
            "wo": np.ascontiguousarray(
                Wo[heads, :, :].reshape(FQ, D), dtype=np.float32),
            "ropeq": rq,
            "ropek": rk,
        })
    return in_maps


def kernel(x, Wq, Wk, Wv, Wo, q_norm_w, k_norm_w):
    in_maps = make_in_maps(x, Wq, Wk, Wv, Wo, q_norm_w, k_norm_w)
    nc = build_nc()
    res = bass_utils.run_bass_kernel_spmd(nc, in_maps, core_ids=list(range(N_CORES)))
    outs = [r["out"] for r in res.results]
    full = np.empty((B, L, D), dtype=np.float32)
    for b in range(BATCH_WAYS):
        full[b] = np.sum(outs[b * HEAD_WAYS:(b + 1) * HEAD_WAYS], axis=0)
    return full


# revision 77
# speedup vs baseline: 1.2900x; 1.0362x over previous
"""Trainium2 Bass kernel for GQA attention block (RMSNorm-qk + RoPE + causal GQA + O-proj).

Problem shapes (hardcoded): B=2, L=2048, D=2048, H=32 q heads, HKV=8 kv heads, HD=64.

Sharding across 8 NeuronCores: 2-way data parallel on batch x 4-way tensor
parallel on heads. Core i handles batch i//4 and head-group i%4 (8 q heads,
2 kv heads). Each core computes its partial output (x[b] @ Wq_s ... @ Wo_s)
of shape [L, D]; the host sums the 4 partials per batch.

Per-core layout / schedule:
  - all activations bf16 (60x rel-err headroom vs the 2e-2 gate), PSUM f32
  - QKV proj: x stationary [dchunk, tok], W moving -> ps_q [tok, 512] /
    ps_kv [tok, 256]
  - RMSNorm inv = exp(-0.5*ln(mean+eps)) so ACT only ever uses
    {Square, Ln, Exp, Copy} = one act-function set -> no table reloads
  - RoPE on DVE in bf16 (2x mode); q heads host-permuted [0,4,1,5,2,6,3,7]
    so the 128-row XBAR DMA-transpose emits qT blocks whose partition halves
    line up with their kv head (no PE transposes at all)
  - attention S^T = kT.T @ qT per 128-key tile into grouped PSUM [P,1024];
    one Exp per group (scale=1/8 folded in); causal diagonal tiles packed
    (512+384 | 256+128) and masked multiplicatively in bf16
  - P@V accumulates vaug.T @ es into ps_o [65, n]; row 64 = softmax denom
    (ones column in vaug); 1/denom broadcast to 64 partitions via a PE outer
    product with a ones stationary, then folded into the PSUM evacuation
  - O-proj: attnT (f32r) stationary, Wo (f32r) moving; wide [P,1024] units
    evacuated alternately on DVE/ACT, one output-row-tile DMA per token tile
  - single preloaded ACT table set; weight/output DMAs on the ACT hwdge
    queue, x stream + XBAR transposes on SP
  - software pipeline per chunk: slot h emits [proj tile of chunk+1] ->
    scores+exp(h) -> PV(h-1)+normalize -> O-proj unit of chunk-1, so PE
    always has matmul work while ACT drains the exps
"""

import sys
from contextlib import nullcontext

import numpy as np

for _p in ("/opt/trn_rl_repo", "/root/.axon_site/_ro/trn_rl_repo"):
    if _p not in sys.path:
        sys.path.append(_p)

import ml_dtypes

import concourse.bass as bass
import concourse.mybir as mybir
import concourse.tile as tile
from concourse import bacc, bass_utils

F32 = mybir.dt.float32
F32R = mybir.dt.float32r
BF16 = mybir.dt.bfloat16
AF = mybir.ActivationFunctionType
AX = mybir.AxisListType
BFNP = ml_dtypes.bfloat16

# full problem shapes
B, L, D = 2, 2048, 2048
H, HKV_TOT, HD = 32, 8, 64
EPS = 1e-5
THETA = 1000000.0

N_CORES = 8
BATCH_WAYS, HEAD_WAYS = 2, 4
HQ = H // HEAD_WAYS          # 8 q heads per core
HKV = HKV_TOT // HEAD_WAYS   # 2 kv heads per core
GQ = HQ // HKV               # 4 q heads per kv head

P = 128
QCW = 512                    # q-chunk width
HW = HD // 2                 # rope half
QPERM = [0, 4, 1, 5, 2, 6, 3, 7]  # host-side q head order (and Wo row order)

NT = L // P                  # 16 token tiles
DC = D // P                  # 16 contraction chunks
NQC = L // QCW               # 4 q chunks
KTQ = QCW // P               # 4 k tiles per q chunk
FQ = HQ * HD                 # 512
FKV = HKV * HD               # 128
SCALE = HD ** -0.5

# es column layout per (head, chunk qc): non-diag kt pairs at 1024*g, then
# packed diag groups A (512+384) and B (256+128)
ES_W = QCW * KTQ * (NQC - 1) + 1280   # 7424

INTERLEAVE = False    # interleave PV matmul pairs between score groups
PROJ_IN_RING = False  # proj PSUM from shared ring (else dedicated pq/pkv banks)
RING_BUFS = 2         # shared-ring depth (3 only if PROJ_IN_RING)
OU_NARROW = False     # 16 narrow O-proj units/chunk (else 8 merged wide)
X_BUFS = 3            # x tile ring depth
EXP_PRIO = 25         # scheduler priority boost for score exps
X_PRIO = 0            # scheduler priority boost for x loads
X_PREFETCH = 4        # x tiles prefetched upfront
OST_PRIO = 25         # priority boost for O-proj evac copies
REC_PRIO = 25         # priority boost for denom reciprocal
RB_PRIO = 0           # priority boost for rb copy + attnT mul
WQKV_GROUP = 1        # wqkv chunks per DMA
EPI_SP = True         # epilogue output stores on SP queue
SC_MM_PRIO = 0        # (negative = deprioritize) score matmul scheduler offset
NRM_PRIO = 0          # priority boost for proj norm chain
EXP_PRIO_LAST = 0     # exp priority override for the last chunk (0 = EXP_PRIO)
ROPE_PRIO = 0         # priority boost for rope ops
VC_PRIO = 0           # priority boost for V copy + XBAR transposes
OST_BUFS = 3          # ost staging buffers
ES_BUFS = 2           # es (softmax weights) buffers
ATT_BUFS = 2          # attnT buffers
NRM_BUFS = 2          # proj norm scratch buffers
MASK_PRIO = 0         # standalone priority for diag mask muls
RB_ON_ACT = False     # rb evac copy on ACT instead of DVE


def es_slice(qc, kt):
    """(offset, width, w0) of k-tile kt inside the es tile for chunk qc."""
    nd = qc * KTQ
    if kt < nd:
        return 1024 * (kt // 2) + QCW * (kt % 2), QCW, 0
    d = kt - nd
    base = KTQ * QCW * qc
    offs = (0, QCW, QCW + 384, QCW + 384 + 256)
    return base + offs[d], QCW - P * d, P * d


def _act_set_id(nc):
    """Index of the act-function set containing everything we use, so one
    preloaded table covers the whole kernel (no mid-kernel reloads).
    Returns None if no single set covers them (fall back to automatic
    table loads, which cost ~40us but stay correct)."""
    try:
        from concourse.hw_specs import get_activation_tables
        need = {AF.Exp, AF.Ln, AF.Square, AF.Copy, AF.Identity}
        for idx, (name, funcs) in enumerate(
                get_activation_tables(nc.m.arch).items()):
            if need <= funcs:
                return idx
    except Exception:
        pass
    return None


def build_nc():
    nc = bacc.Bacc("TRN2", target_bir_lowering=False, debug=False)

    xt = nc.dram_tensor("xt", [NT, P, DC, P], BF16, kind="ExternalInput").ap()
    wqkv = nc.dram_tensor("wqkv", [D, FQ + 2 * FKV], BF16, kind="ExternalInput").ap()
    wo = nc.dram_tensor("wo", [FQ, D], F32R, kind="ExternalInput").ap()
    ropeq = nc.dram_tensor("ropeq", [P, NT, 4, HW], BF16, kind="ExternalInput").ap()
    ropek = nc.dram_tensor("ropek", [P, NT, 4, HW], BF16, kind="ExternalInput").ap()
    out = nc.dram_tensor("out", [L, D], F32, kind="ExternalOutput").ap()

    with tile.TileContext(nc) as tc:
        with (
            tc.tile_pool(name="consts", bufs=1) as consts,
            tc.tile_pool(name="weights", bufs=1) as weights,
            tc.tile_pool(name="persist", bufs=1) as persist,
            tc.tile_pool(name="xin", bufs=2) as xin,
            tc.tile_pool(name="nrm", bufs=2) as nrm,
            tc.tile_pool(name="esb", bufs=2) as esb,
            tc.tile_pool(name="attnp", bufs=2) as attnp,
            tc.tile_pool(name="evacp", bufs=1) as evacp,
            tc.tile_pool(name="ps_s", bufs=RING_BUFS, space="PSUM") as ps_s_pool,
            tc.tile_pool(name="ps_proj", bufs=1, space="PSUM") as ps_proj,
            tc.tile_pool(name="ps_o", bufs=2, space="PSUM") as ps_o_pool,
        ):
            # ---------- constants ----------
            act_set = _act_set_id(nc)
            if act_set is not None:
                nc.scalar.add_instruction(mybir.InstLoadActFuncSet(
                    name=nc.get_next_instruction_name(), ins=[], outs=[],
                    act_func_set_id=act_set))
            eps_sb = consts.tile([P, 1], F32)
            nc.vector.memset(eps_sb, EPS)
            ones_bf = consts.tile([P, 1], BF16)
            nc.vector.memset(ones_bf, 1.0)
            ones_st = consts.tile([1, HD], BF16)
            nc.vector.memset(ones_st, 1.0)
            # packed diagonal masks: per 128-key tile, column j valid iff j >= p
            maskA = consts.tile([P, QCW + 384], BF16)
            maskB = consts.tile([P, 256 + P], BF16)
            for m, ws in ((maskA, (QCW, 384)), (maskB, (256, P))):
                nc.vector.memset(m, 1.0)
                off = 0
                for w in ws:
                    nc.gpsimd.affine_select(
                        out=m[:, off:off + w], in_=m[:, off:off + w],
                        pattern=[[1, w]], compare_op=mybir.AluOpType.is_ge,
                        fill=0.0, base=0, channel_multiplier=-1,
                    )
                    off += w

            # ---------- x prefetch ----------
            xin_next = {}

            def load_x(t):
                x_sb = xin.tile([P, DC, P], BF16, name="x_sb", tag="x_sb", bufs=X_BUFS)
                with tc.high_priority(offset=X_PRIO) if X_PRIO else nullcontext():
                    nc.sync.dma_start(out=x_sb, in_=xt[t])
                return x_sb

            # weight/table loads go through the ACT hwdge queue so the
            # latency-critical x stream owns the SP queue
            wqkv_sb = weights.tile([P, DC, FQ + 2 * FKV], BF16)
            wq_r = wqkv.rearrange("(c p) j -> p c j", p=P)
            nc.scalar.dma_start(
                out=wqkv_sb[:, 0:WQKV_GROUP, :], in_=wq_r[:, 0:WQKV_GROUP, :])
            for _t in range(min(X_BUFS, X_PREFETCH)):
                xin_next[_t] = load_x(_t)
            for c in range(WQKV_GROUP, DC, WQKV_GROUP):
                nc.scalar.dma_start(
                    out=wqkv_sb[:, c:c + WQKV_GROUP, :],
                    in_=wq_r[:, c:c + WQKV_GROUP, :])
            rq = consts.tile([P, NT, 4, HW], BF16)
            nc.scalar.dma_start(out=rq, in_=ropeq)
            rk = consts.tile([P, NT, 4, HW], BF16)
            nc.scalar.dma_start(out=rk, in_=ropek)

            # ---------- persistent activations ----------
            # qT block j holds q heads j (parts 0:64) and j+4 (parts 64:128)
            qT = persist.tile([P, HQ // 2, L], BF16)
            kT = persist.tile([P, L], BF16)  # kv0 parts 0:64, kv1 parts 64:128
            vaug = persist.tile([P, NT, HKV, HD + 1], BF16)
            nc.vector.tensor_copy(
                vaug[:, :, :, HD:HD + 1],
                ones_bf.unsqueeze(2).unsqueeze(3).to_broadcast([P, NT, HKV, 1]))

            # wo needed first ~100us in; DMAs are emitted mid-prologue (below)
            wo_sb = weights.tile([P, FQ // P, D], F32R)

            def load_wo():
                for c in range(FQ // P):
                    nc.scalar.dma_start(
                        out=wo_sb[:, c, :],
                        in_=wo.rearrange("(c p) j -> p c j", p=P)[:, c, :])

            def q_ap(h):
                off = 0 if h < GQ else HD
                return qT[off:off + HD, h % GQ, :]

            def k_ap(kv):
                return kT[kv * HD:(kv + 1) * HD, :]

            # ---------- projection + norm + rope for one token tile ----------
            def proj_tile(t):
                x_sb = xin_next.pop(t, None)
                if x_sb is None:
                    x_sb = load_x(t)
                for tn in (t + 1, t + 2):
                    if tn < NT and tn not in xin_next:
                        xin_next[tn] = load_x(tn)

                if PROJ_IN_RING:
                    ps_q = ps_s_pool.tile(
                        [P, 1024], F32, name="ps_q", tag="ps",
                        bufs=RING_BUFS)[:, 0:FQ]
                else:
                    ps_q = ps_proj.tile([P, FQ], F32, name="ps_q", tag="pq", bufs=1)
                for c in range(DC):
                    nc.tensor.matmul(
                        ps_q, x_sb[:, c, :], wqkv_sb[:, c, 0:FQ],
                        start=(c == 0), stop=(c == DC - 1))
                if PROJ_IN_RING:
                    ps_kv = ps_s_pool.tile(
                        [P, 1024], F32, name="ps_kv", tag="ps",
                        bufs=RING_BUFS)[:, 0:2 * FKV]
                else:
                    ps_kv = ps_proj.tile(
                        [P, 2 * FKV], F32, name="ps_kv", tag="pkv", bufs=1)
                for c in range(DC):
                    nc.tensor.matmul(
                        ps_kv, x_sb[:, c, :], wqkv_sb[:, c, FQ:FQ + 2 * FKV],
                        start=(c == 0), stop=(c == DC - 1))

                psq_g = ps_q.rearrange("p (h e) -> p h e", e=HD)
                psk_g = ps_kv[:, 0:FKV].rearrange("p (h e) -> p h e", e=HD)
                nh = HQ + HKV  # 10 normed heads (q then k)

                sq = nrm.tile([P, nh, HD], BF16, name="sq", tag="sq", bufs=NRM_BUFS)
                with tc.high_priority(offset=NRM_PRIO) if NRM_PRIO                         else nullcontext():
                    nc.scalar.activation(sq[:, 0:HQ, :], psq_g, AF.Square)
                    nc.scalar.activation(sq[:, HQ:nh, :], psk_g, AF.Square)
                ss = nrm.tile([P, nh], F32, name="ss", tag="ss", bufs=NRM_BUFS)
                nc.vector.reduce_sum(out=ss, in_=sq, axis=AX.X)
                lnt = nrm.tile([P, nh], F32, name="lnt", tag="lnt", bufs=NRM_BUFS)
                nc.scalar.activation(lnt, ss, AF.Ln, scale=1.0 / HD, bias=eps_sb)
                inv = nrm.tile([P, nh], F32, name="inv", tag="inv", bufs=NRM_BUFS)
                nc.scalar.activation(inv, lnt, AF.Exp, scale=-0.5)

                qn = nrm.tile([P, nh, HD], BF16, name="qn", tag="qn", bufs=NRM_BUFS)
                nc.vector.tensor_mul(
                    qn[:, 0:HQ, :], psq_g,
                    inv[:, 0:HQ].unsqueeze(2).to_broadcast([P, HQ, HD]))
                nc.vector.tensor_mul(
                    qn[:, HQ:nh, :], psk_g,
                    inv[:, HQ:nh].unsqueeze(2).to_broadcast([P, HKV, HD]))

                qr = nrm.tile([P, nh, HD], BF16, name="qr", tag="qr", bufs=NRM_BUFS)
                tmp = nrm.tile([P, nh, HW], BF16, name="tmp", tag="tmp", bufs=NRM_BUFS)
                rope_ctx = (tc.high_priority(offset=ROPE_PRIO) if ROPE_PRIO
                            else nullcontext())
                rope_ctx.__enter__()
                for (a, b, rt) in ((0, HQ, rq), (HQ, nh, rk)):
                    n = b - a

                    def tab(i):
                        return rt[:, t, i, :].unsqueeze(1).to_broadcast([P, n, HW])

                    q1, q2 = qn[:, a:b, 0:HW], qn[:, a:b, HW:HD]
                    r1, r2 = qr[:, a:b, 0:HW], qr[:, a:b, HW:HD]
                    tm = tmp[:, a:b, :]
                    # r1 = q1*C1 - q2*S2 ; r2 = q2*C2 + q1*S1
                    nc.vector.tensor_mul(r1, q1, tab(0))
                    nc.vector.tensor_mul(tm, q2, tab(3))
                    nc.vector.tensor_sub(r1, r1, tm)
                    nc.vector.tensor_mul(r2, q2, tab(2))
                    nc.vector.tensor_mul(tm, q1, tab(1))
                    nc.vector.tensor_add(r2, r2, tm)

                rope_ctx.__exit__(None, None, None)
                with tc.high_priority(offset=VC_PRIO) if VC_PRIO else nullcontext():
                    # XBAR transposes: qr [128,512] -> qT[:, :, t*128:+128]
                    nc.sync.dma_start_transpose(
                        qT[:, :, t * P:(t + 1) * P], qr[:, 0:HQ, :])
                    nc.sync.dma_start_transpose(
                        kT[:, t * P:(t + 1) * P], qr[:, HQ:nh, :])
                    # V (un-normed, un-roped)
                    nc.scalar.copy(
                        vaug[:, t, :, 0:HD],
                        ps_kv[:, FKV:2 * FKV].rearrange("p (h e) -> p h e", e=HD))

            # ---------- attention: scores(h) interleaved with P@V(h-1) ----------
            # PE emits score-groups at ~0.4us while ACT drains each exp in
            # ~0.9us; raw emission outruns the 3-deep PSUM ring. Interleaving
            # PV matmul pairs of the previous head between score groups
            # rate-matches PE to ACT.
            def score_groups(qc, h, es):
                qs = q_ap(h)
                ks = k_ap(h // GQ)
                nd = qc * KTQ
                q0 = qc * QCW
                base = KTQ * QCW * qc

                def mm(ps, pcol, kt, qoff, w, start, stop):
                    with tc.high_priority(offset=SC_MM_PRIO) if SC_MM_PRIO                             else nullcontext():
                        nc.tensor.matmul(
                            ps[:, pcol:pcol + w],
                            ks[:, kt * P:(kt + 1) * P],
                            qs[:, q0 + qoff:q0 + qoff + w],
                            start=start, stop=stop)

                eprio = (EXP_PRIO_LAST if (EXP_PRIO_LAST and qc == NQC - 1)
                         else EXP_PRIO)

                def g_nd(g):
                    def f():
                        ps = ps_s_pool.tile(
                            [P, 1024], F32, name="ps_s", tag="ps", bufs=RING_BUFS)
                        mm(ps, 0, 2 * g, 0, QCW, True, True)
                        mm(ps, QCW, 2 * g + 1, 0, QCW, True, True)
                        off = 1024 * g
                        with tc.high_priority(offset=eprio):
                            nc.scalar.activation(
                                es[:, off:off + 1024], ps[:, 0:1024], AF.Exp,
                                scale=SCALE)
                    return f

                def g_diag(dgl):
                    def f():
                        ps = ps_s_pool.tile(
                            [P, 1024], F32, name="ps_s", tag="ps", bufs=RING_BUFS)
                        if dgl == 0:  # A = (d0 512 | d1 384)
                            mm(ps, 0, nd, 0, QCW, True, True)
                            mm(ps, QCW, nd + 1, P, 384, True, True)
                            eo, w, mask = base, 896, maskA
                        else:         # B = (d2 256 | d3 128)
                            mm(ps, 0, nd + 2, 2 * P, 256, True, True)
                            mm(ps, 256, nd + 3, 3 * P, P, True, True)
                            eo, w, mask = base + 896, 384, maskB
                        with tc.high_priority(offset=eprio):
                            nc.scalar.activation(
                                es[:, eo:eo + w], ps[:, 0:w], AF.Exp, scale=SCALE)
                        with tc.high_priority(offset=MASK_PRIO) if MASK_PRIO                                 else nullcontext():
                            nc.vector.tensor_mul(
                                es[:, eo:eo + w], es[:, eo:eo + w], mask)
                    return f

                return [g_nd(g) for g in range(nd // 2)] + [g_diag(0), g_diag(1)]

            def pv_pairs(qc, h, es, ps_o):
                kv = h // GQ
                nkt = (qc + 1) * KTQ

                def pair(k0):
                    def f():
                        for kt in (k0, k0 + 1):
                            off, w, w0 = es_slice(qc, kt)
                            nc.tensor.matmul(
                                ps_o[:, w0:QCW], vaug[:, kt, kv, :],
                                es[:, off:off + w],
                                start=(kt == 0), stop=(kt == nkt - 1))
                    return f

                return [pair(k0) for k0 in range(0, nkt, 2)]

            def pv_rec(ps_o):
                rec = evacp.tile([1, QCW], BF16, name="rec", tag="rec", bufs=4)
                with tc.high_priority(offset=REC_PRIO) if REC_PRIO else nullcontext():
                    with nc.allow_low_precision("1/denom bf16; 2e-2 budget"):
                        nc.vector.reciprocal(rec, ps_o[HD:HD + 1, :])
                return rec

            def pv_finish(h, ps_o, rec, attnT):
                # broadcast 1/denom to 64 partitions via PE outer product
                ps_r = ps_s_pool.tile([P, 1024], F32, name="ps_r", tag="ps", bufs=RING_BUFS)
                nc.tensor.matmul(
                    ps_r[0:HD, 0:QCW], ones_st, rec, start=True, stop=True)
                rb = evacp.tile([HD, QCW], F32, name="rb", tag="rb", bufs=4)
                with tc.high_priority(offset=RB_PRIO) if RB_PRIO else nullcontext():
                    if RB_ON_ACT:
                        nc.scalar.copy(rb, ps_r[0:HD, 0:QCW])
                    else:
                        nc.vector.tensor_copy(rb, ps_r[0:HD, 0:QCW])
                    off = 0 if h < GQ else HD
                    nc.vector.tensor_mul(
                        attnT[off:off + HD, h % GQ, :], ps_o[0:HD, :], rb)

            def attn_slot(qc, h, es_pv, attnT=None, interleave=INTERLEAVE):
                """scores+exp head h; PV+evac of head h-1."""
                es = esb.tile([P, ES_W], BF16, name="es", tag="es", bufs=ES_BUFS)
                sc = score_groups(qc, h, es)
                pv = []
                ps_o = None
                if es_pv is not None:
                    ps_o = ps_o_pool.tile(
                        [HD + 1, QCW], F32, name="ps_o", tag="po", bufs=2)
                    pv = pv_pairs(qc, h - 1, es_pv, ps_o)
                if interleave:
                    order = sc[0:2] + [x for p in zip(pv, sc[2:]) for x in p]
                    n = min(len(pv), len(sc) - 2)
                    order += pv[n:] + sc[2 + n:]
                else:
                    order = sc + pv
                for f in order:
                    f()
                if es_pv is not None:
                    pv_finish(h - 1, ps_o, pv_rec(ps_o), attnT)
                return es

            def pv_head(qc, h, es):
                ps_o = ps_o_pool.tile(
                    [HD + 1, QCW], F32, name="ps_o", tag="po", bufs=2)
                for f in pv_pairs(qc, h, es, ps_o):
                    f()
                return (h, ps_o, pv_rec(ps_o))

            # ---------- normalize into attnT, two slots later (Pool, SBUF-only) ----
            # ---------- O-projection unit: one token tile, two column chunks ----------
            ost_cur = [None]

            def oproj_unit(qc, u, attnT, epilogue=False):
                if OU_NARROW:
                    # u in 0..15 per chunk: one column chunk per unit
                    tt, col = u // KTQ, u % KTQ
                    ps = ps_s_pool.tile(
                        [P, 1024], F32, name="ps_s", tag="ps", bufs=RING_BUFS)
                    for fc in range(FQ // P):
                        nc.tensor.matmul(
                            ps[:, 0:QCW],
                            attnT[:, fc, tt * P:(tt + 1) * P],
                            wo_sb[:, fc, col * QCW:(col + 1) * QCW],
                            start=(fc == 0), stop=(fc == FQ // P - 1))
                    ost = evacp.tile([P, QCW], F32, name="ostn", tag="ostn", bufs=3)
                    if u % 2 == 0:
                        nc.vector.tensor_copy(ost, ps[:, 0:QCW])
                    else:
                        nc.scalar.copy(ost, ps[:, 0:QCW])
                    row0 = qc * QCW + tt * P
                    nc.scalar.dma_start(
                        out=out[row0:row0 + P, col * QCW:(col + 1) * QCW],
                        in_=ost)
                    return
                # u in 0..7 per chunk: token tile u//2, column pair u%2
                tt, cp = u // 2, u % 2
                ps = ps_s_pool.tile([P, 1024], F32, name="ps_s", tag="ps", bufs=RING_BUFS)
                for half in range(2):
                    col = 2 * cp + half
                    for fc in range(FQ // P):
                        nc.tensor.matmul(
                            ps[:, half * QCW:(half + 1) * QCW],
                            attnT[:, fc, tt * P:(tt + 1) * P],
                            wo_sb[:, fc, col * QCW:(col + 1) * QCW],
                            start=(fc == 0), stop=(fc == FQ // P - 1))
                if cp == 0:
                    ost_cur[0] = evacp.tile(
                        [P, D], F32, name="ost", tag="ost", bufs=OST_BUFS)
                ost = ost_cur[0]
                with tc.high_priority(offset=OST_PRIO) if OST_PRIO else nullcontext():
                    if u % 2 == 0:
                        nc.vector.tensor_copy(ost[:, 0:2 * QCW], ps)
                    else:
                        nc.scalar.copy(ost[:, 2 * QCW:4 * QCW], ps)
                if cp == 1:
                    row0 = qc * QCW + tt * P
                    eng = nc.sync if (EPI_SP and epilogue) else nc.scalar
                    eng.dma_start(out=out[row0:row0 + P, :], in_=ost)

            # ---------- software-pipelined schedule ----------
            for t in range(KTQ):
                proj_tile(t)
            attnT_cur = None
            attnT_prev = None
            for qc in range(NQC):
                attnT_prev = attnT_cur
                attnT_cur = attnp.tile(
                    [P, FQ // P, QCW], F32R, name="attnT", tag="attnT", bufs=ATT_BUFS)
                es_prev = None
                for h in range(HQ):
                    if qc == 0 and h == 2:
                        load_wo()
                    if qc + 1 < NQC and h % 2 == 0:
                        proj_tile(KTQ * (qc + 1) + h // 2)
                    es_prev = attn_slot(qc, h, es_prev, attnT_cur)
                    if qc > 0:
                        if OU_NARROW:
                            oproj_unit(qc - 1, 2 * h, attnT_prev)
                            oproj_unit(qc - 1, 2 * h + 1, attnT_prev)
                        else:
                            oproj_unit(qc - 1, h, attnT_prev)
                h, ps_o, rec = pv_head(qc, HQ - 1, es_prev)
                pv_finish(h, ps_o, rec, attnT_cur)
            for u in range((4 if OU_NARROW else 2) * KTQ):
                oproj_unit(NQC - 1, u, attnT_cur, epilogue=True)

    nc.compile()
    return nc


def make_rope_tables(norm_w, l, nt):
    """[P, nt, 4, 32] bf16: C1=cos*w1, S1=sin*w1, C2=cos*w2, S2=sin*w2."""
    inv_freq = THETA ** (-np.arange(0, HD, 2, dtype=np.float64) / HD)
    ang = np.arange(l, dtype=np.float64)[:, None] * inv_freq[None, :]
    cos, sin = np.cos(ang), np.sin(ang)  # [l, 32]
    w1 = norm_w[:HW].astype(np.float64)
    w2 = norm_w[HW:].astype(np.float64)
    tabs = np.stack([cos * w1, sin * w1, cos * w2, sin * w2], axis=1)  # [l,4,32]
    return np.ascontiguousarray(
        tabs.reshape(nt, P, 4, HW).transpose(1, 0, 2, 3)).astype(BFNP)


def make_in_maps(x, Wq, Wk, Wv, Wo, q_norm_w, k_norm_w):
    rq = make_rope_tables(np.asarray(q_norm_w), L, NT)
    rk = make_rope_tables(np.asarray(k_norm_w), L, NT)
    Wq = np.asarray(Wq, np.float32).reshape(D, H, HD)
    Wo = np.asarray(Wo, np.float32).reshape(H, HD, D)
    in_maps = []
    for i in range(N_CORES):
        b, g = i // HEAD_WAYS, i % HEAD_WAYS
        heads = [g * HQ + p for p in QPERM]
        wq_s = Wq[:, heads, :].reshape(D, FQ)
        wk_s = np.asarray(Wk, np.float32)[:, g * FKV:(g + 1) * FKV]
        wv_s = np.asarray(Wv, np.float32)[:, g * FKV:(g + 1) * FKV]
        xb = np.asarray(x[b], np.float32).astype(BFNP)
        # xt[t, p, c, j] = x[b][t*128 + j, c*128 + p]
        xtl = np.ascontiguousarray(
            xb.reshape(NT, P, DC, P).transpose(0, 3, 2, 1))
        in_maps.append({
            "xt": xtl,
            "wqkv": np.ascontiguousarray(
                np.concatenate([wq_s, wk_s, wv_s], axis=1)).astype(BFNP),
            "wo": np.ascontiguousarray(
                Wo[heads, :, :].reshape(FQ, D), dtype=np.float32),
            "ropeq": rq,
            "ropek": rk,
        })
    return in_maps


def kernel(x, Wq, Wk, Wv, Wo, q_norm_w, k_norm_w):
    in_maps = make_in_maps(x, Wq, Wk, Wv, Wo, q_norm_w, k_norm_w)
    nc = build_nc()
    res = bass_utils.run_bass_kernel_spmd(nc, in_maps, core_ids=list(range(N_CORES)))
    outs = [r["out"] for r in res.results]
    full = np.empty((B, L, D), dtype=np.float32)
    for b in range(BATCH_WAYS):
        full[b] = np.sum(outs[b * HEAD_WAYS:(b + 1) * HEAD_WAYS], axis=0)
    return full
